# revision 12
# baseline (speedup 1.0000x reference)
"""Trainium2 Bass kernel for EvolvedLoopLinear: out = x @ W.T + b.

Full shapes: x [4096, 4096] f32, W [4096, 4096] f32, b [4096] f32.
Sharding: 2D over 8 cores — batch split 4 ways, out_dim split 2 ways.
Per core: out_T[n, m] = sum_k W[n, k] * x[m, k] + b[n] with
  M = 1024 batch rows, N = 2048 out cols, K = 4096 contraction.
The output is computed transposed (out_dim on PSUM partitions) so the
per-partition bias rides the ACT-engine PSUM->SBUF eviction for free.
"""

import math
import sys

for _p in ("/opt/trn_rl_repo",):
    if _p not in sys.path:
        sys.path.insert(0, _p)

import numpy as np

import concourse.bass as bass  # noqa: F401  (registers AP machinery)
import concourse.mybir as mybir
import concourse.tile as tile
from concourse import bacc
from concourse.bass_utils import run_bass_kernel_spmd

BATCH = 4096
IN_DIM = 4096
OUT_DIM = 4096
N_CORES = 8
M_SHARD = 4  # batch split
N_SHARD = 2  # out_dim split
M = BATCH // M_SHARD  # 1024 batch rows per core
N = OUT_DIM // N_SHARD  # 2048 out cols per core
P = 128
KO = IN_DIM // P  # 32 contraction subtiles
NSUB = N // P  # 16 out-partition blocks
MT = 512  # PSUM free dim per tile
MTILES = M // MT  # 2

_CACHE: dict = {}


def _build_program(
    repeats: int = 1,
    mode: str = "bf16",
    out_engine: str = "gpsimd",
    xchunk: int = 2,
    w_split: int = 2,
    w_bufs: int = 8,
    x_engine: str = "scalar",
    w_engine: str = "sync",
    out_bufs: int = 4,
    unroll: int = 4,
    x_bufs: int = 2,
):
    """Emit + compile the per-core SPMD program (identical on all cores).

    repeats > 1 wraps the whole body in a dynamic For_i loop — used only
    for steady-state timing (the body is idempotent)."""
    nc = bacc.Bacc("TRN2", target_bir_lowering=False, debug=False, num_devices=N_CORES)
    dt = {
        "fp32r": mybir.dt.float32r,
        "fp32": mybir.dt.float32,
        "bf16": mybir.dt.bfloat16,
    }[mode]
    xt = nc.declare_dram_parameter("xt", [P, KO, M], dt, isOutput=False)
    wt = nc.declare_dram_parameter("wt", [P, NSUB, KO, P], dt, isOutput=False)
    bs = nc.declare_dram_parameter("bs", [P, NSUB], mybir.dt.float32, isOutput=False)
    ot = nc.declare_dram_parameter("ot", [P, NSUB, M], mybir.dt.float32, isOutput=True)

    with tile.TileContext(nc) as tc:
        with (
            tc.tile_pool(name="xres", bufs=x_bufs) as xres_pool,
            tc.tile_pool(name="wblk", bufs=w_bufs) as w_pool,
            tc.tile_pool(name="psum", bufs=8, space="PSUM") as psum_pool,
            tc.tile_pool(name="outp", bufs=out_bufs) as out_pool,
            tc.tile_pool(name="bias", bufs=1) as b_pool,
        ):

            def body(_iv=None):
                bias_sb = b_pool.tile([P, NSUB], mybir.dt.float32)
                nc.sync.dma_start(bias_sb[:], bs[:])

                # x shard stays SBUF-resident (16 MB); load in KO-chunks
                # so compute can start before the whole shard lands.
                xres = xres_pool.tile([P, KO, M], dt)
                XCHUNK = xchunk
                x_dma = getattr(nc, x_engine)
                for kc in range(0, KO, XCHUNK):
                    x_dma.dma_start(
                        xres[:, kc : kc + XCHUNK], xt[:, kc : kc + XCHUNK]
                    )

                # W streams in half-K blocks (8 KB/partition) with deep
                # buffering so the next block's DMA hides under compute.
                KHALF = KO // w_split
                for ns in range(NSUB):
                    whs = [
                        w_pool.tile([P, KHALF, P], dt, name=f"wh{i}", tag="wh")
                        for i in range(w_split)
                    ]
                    w_dma = getattr(nc, w_engine)
                    for i in range(w_split):
                        w_dma.dma_start(
                            whs[i][:], wt[:, ns, i * KHALF : (i + 1) * KHALF]
                        )
                    pts = [
                        psum_pool.tile([P, MT], mybir.dt.float32, name=f"pt{i}", tag="pt")
                        for i in range(MTILES)
                    ]
                    # ko outer / mt inner: consecutive matmuls share the
                    # stationary W block, halving LDWEIGHTS traffic.
                    for ko in range(KO):
                        for mt in range(MTILES):
                            nc.tensor.matmul(
                                pts[mt][:],
                                whs[ko // KHALF][:, ko % KHALF],
                                xres[:, ko, mt * MT : (mt + 1) * MT],
                                start=(ko == 0),
                                stop=(ko == KO - 1),
                            )
                    for mt in range(MTILES):
                        ot_sb = out_pool.tile([P, MT], mybir.dt.float32)
                        nc.scalar.add(ot_sb[:], pts[mt][:], bias_sb[:, ns : ns + 1])
                        out_dma = nc.gpsimd if out_engine == "gpsimd" else nc.sync
                        out_dma.dma_start(
                            ot[:, ns, mt * MT : (mt + 1) * MT], ot_sb[:]
                        )

            if repeats == 1:
                body()
            else:
                # Unroll several complete forward passes per For_i trip:
                # For_i inserts an all-engine barrier at each back edge, so
                # unrolling amortizes the barrier + pipeline refill, and the
                # x_bufs=2 ping-pong lets body k+1's x DMA stream overlap
                # body k's tail matmuls.
                u = math.gcd(unroll, repeats)
                with tc.For_i(0, repeats // u, 1) as iv:
                    for _ in range(u):
                        body(iv)

    nc.compile()
    _dedupe_ldweights(nc)
    return nc


def _dedupe_ldweights(nc) -> int:
    """Drop redundant back-to-back InstLdweights from the compiled PE streams.

    The lowering splits every matmul into its own LDWEIGHTS + MATMUL
    (ldweights=False) pair, even when consecutive matmuls share the same
    stationary block, and the PE executes each LDWEIGHTS serially
    (~50 ns per 128-col load, measured).  The PE array keeps its weights
    across matmuls that don't self-load, so an LDWEIGHTS whose operand is
    identical to the previous one (and that carries no semaphore wait or
    update) is pure overhead.  Removing them saves ~matmul_count/2 loads.
    """
    removed = 0
    for fn in nc.m.functions:
        for blk in fn.blocks:
            prev_key = None
            keep = []
            for inst in blk.instructions:
                if getattr(inst, "engine", None) != mybir.EngineType.PE:
                    keep.append(inst)
                    continue
                nm = inst.__class__.__name__
                if nm == "InstLdweights":
                    key = (
                        str(inst.ins[0]),
                        inst.is_transpose,
                        inst.tile_position,
                        inst.perf_mode,
                        str(inst.tile_size),
                    )
                    if (
                        key == prev_key
                        and not inst.has_wait()
                        and not inst.has_update()
                    ):
                        removed += 1
                        continue
                    prev_key = key
                    keep.append(inst)
                elif nm == "InstMatmult" and inst.ldweights is False:
                    # does not disturb the loaded weights
                    keep.append(inst)
                else:
                    # anything else on PE: conservatively invalidate
                    prev_key = None
                    keep.append(inst)
            if removed:
                blk.instructions[:] = keep
    return removed


def _shard_inputs(x: np.ndarray, W: np.ndarray, b: np.ndarray, mode: str = "bf16"):
    """Host-side shard + retile into the DMA-friendly layouts."""
    import ml_dtypes

    np_dt = ml_dtypes.bfloat16 if mode == "bf16" else np.float32
    in_maps = []
    wt_cache = {}
    bs_cache = {}
    for c in range(N_CORES):
        q, h = divmod(c, N_SHARD)
        xs = x[q * M : (q + 1) * M]  # [M, IN]
        xt = xs.reshape(M, KO, P).transpose(2, 1, 0).astype(np_dt, order="C")
        if (h,) not in wt_cache:
            Ws = W[h * N : (h + 1) * N]  # [N, IN]
            wt_cache[(h,)] = (
                Ws.reshape(NSUB, P, KO, P)
                .transpose(3, 0, 2, 1)
                .astype(np_dt, order="C")
            )
            bs_cache[(h,)] = np.ascontiguousarray(
                b[h * N : (h + 1) * N].reshape(NSUB, P).T
            )
        in_maps.append({"xt": xt, "wt": wt_cache[(h,)], "bs": bs_cache[(h,)]})
    return in_maps


def _assemble(results) -> np.ndarray:
    out = np.empty((BATCH, OUT_DIM), dtype=np.float32)
    for c in range(N_CORES):
        q, h = divmod(c, N_SHARD)
        ot = results[c]["ot"]  # [P, NSUB, M]
        block = ot.transpose(2, 1, 0).reshape(M, N)
        out[q * M : (q + 1) * M, h * N : (h + 1) * N] = block
    return out


def kernel(x: np.ndarray, W: np.ndarray, b: np.ndarray) -> np.ndarray:
    x = np.asarray(x, dtype=np.float32)
    W = np.asarray(W, dtype=np.float32)
    b = np.asarray(b, dtype=np.float32)
    assert x.shape == (BATCH, IN_DIM) and W.shape == (OUT_DIM, IN_DIM)

    if "nc" not in _CACHE:
        _CACHE["nc"] = _build_program()
    nc = _CACHE["nc"]

    in_maps = _shard_inputs(x, W, b)
    res = run_bass_kernel_spmd(nc, in_maps, list(range(N_CORES)))
    return _assemble(res.results)


if __name__ == "__main__":
    rng = np.random.default_rng(0)
    x = rng.standard_normal((BATCH, IN_DIM), dtype=np.float32)
    W = rng.uniform(-1 / 64, 1 / 64, size=(OUT_DIM, IN_DIM)).astype(np.float32)
    b = rng.uniform(-1 / 64, 1 / 64, size=(OUT_DIM,)).astype(np.float32)
    got = kernel(x, W, b)
    exp = x @ W.T + b
    scale = np.abs(exp).max()
    print("absmax err:", np.abs(got - exp).max(), "scale:", scale)



# revision 16
# speedup vs baseline: 1.0029x; 1.0029x over previous
"""Trainium2 Bass kernel for EvolvedLoopLinear: out = x @ W.T + b.

Full shapes: x [4096, 4096] f32, W [4096, 4096] f32, b [4096] f32.
Sharding: 2D over 8 cores — batch split 4 ways, out_dim split 2 ways.
Per core: out_T[n, m] = sum_k W[n, k] * x[m, k] + b[n] with
  M = 1024 batch rows, N = 2048 out cols, K = 4096 contraction.
The output is computed transposed (out_dim on PSUM partitions) so the
per-partition bias rides the ACT-engine PSUM->SBUF eviction for free.

x and W are cast to bf16 on the host (PE runs bf16 at full rate where
fp32r pays slower weight loads; PSUM accumulation and the bias add stay
fp32; measured rel err 2.1e-3 vs the f32 reference, gate is 2e-2).
x stays SBUF-resident per body and double-buffered across bodies
(x_bufs=2) so the next forward's x stream overlaps the current tail
matmuls; W streams in half-K blocks with a 3-block prefetch ring.
For timing loops, `unroll` complete forward passes are emitted per
For_i trip to amortize the loop's all-engine barrier (keep the PE
instruction count per trip under ~8K: unroll=8 wedged the exec unit).
"""

import math
import sys

for _p in ("/opt/trn_rl_repo",):
    if _p not in sys.path:
        sys.path.insert(0, _p)

import numpy as np

import concourse.bass as bass  # noqa: F401  (registers AP machinery)
import concourse.mybir as mybir
import concourse.tile as tile
from concourse import bacc
from concourse.bass_utils import run_bass_kernel_spmd

BATCH = 4096
IN_DIM = 4096
OUT_DIM = 4096
N_CORES = 8
M_SHARD = 4  # batch split
N_SHARD = 2  # out_dim split
M = BATCH // M_SHARD  # 1024 batch rows per core
N = OUT_DIM // N_SHARD  # 2048 out cols per core
P = 128
KO = IN_DIM // P  # 32 contraction subtiles
NSUB = N // P  # 16 out-partition blocks
MT = 512  # PSUM free dim per tile
MTILES = M // MT  # 2

_CACHE: dict = {}


def _build_program(
    repeats: int = 1,
    mode: str = "bf16",
    out_engine: str = "gpsimd",
    xchunk: int = 2,
    w_split: int = 2,
    w_bufs: int = 6,
    x_engine: str = "scalar",
    w_engine: str = "sync",
    out_bufs: int = 4,
    unroll: int = 4,
    x_bufs: int = 2,
    dedupe_ldw: bool = False,
):
    """Emit + compile the per-core SPMD program (identical on all cores).

    repeats > 1 wraps the whole body in a dynamic For_i loop — used only
    for steady-state timing (the body is idempotent)."""
    nc = bacc.Bacc("TRN2", target_bir_lowering=False, debug=False, num_devices=N_CORES)
    dt = {
        "fp32r": mybir.dt.float32r,
        "fp32": mybir.dt.float32,
        "bf16": mybir.dt.bfloat16,
    }[mode]
    xt = nc.declare_dram_parameter("xt", [P, KO, M], dt, isOutput=False)
    wt = nc.declare_dram_parameter("wt", [P, NSUB, KO, P], dt, isOutput=False)
    bs = nc.declare_dram_parameter("bs", [P, NSUB], mybir.dt.float32, isOutput=False)
    ot = nc.declare_dram_parameter("ot", [P, NSUB, M], mybir.dt.float32, isOutput=True)

    with tile.TileContext(nc) as tc:
        with (
            tc.tile_pool(name="xres", bufs=x_bufs) as xres_pool,
            tc.tile_pool(name="wblk", bufs=w_bufs) as w_pool,
            tc.tile_pool(name="psum", bufs=8, space="PSUM") as psum_pool,
            tc.tile_pool(name="outp", bufs=out_bufs) as out_pool,
            tc.tile_pool(name="bias", bufs=1) as b_pool,
        ):

            def body(_iv=None):
                bias_sb = b_pool.tile([P, NSUB], mybir.dt.float32)
                nc.sync.dma_start(bias_sb[:], bs[:])

                # x shard stays SBUF-resident (16 MB); load in KO-chunks
                # so compute can start before the whole shard lands.
                xres = xres_pool.tile([P, KO, M], dt)
                XCHUNK = xchunk
                x_dma = getattr(nc, x_engine)
                for kc in range(0, KO, XCHUNK):
                    x_dma.dma_start(
                        xres[:, kc : kc + XCHUNK], xt[:, kc : kc + XCHUNK]
                    )

                # W streams in half-K blocks (8 KB/partition) with deep
                # buffering so the next block's DMA hides under compute.
                KHALF = KO // w_split
                for ns in range(NSUB):
                    whs = [
                        w_pool.tile([P, KHALF, P], dt, name=f"wh{i}", tag="wh")
                        for i in range(w_split)
                    ]
                    w_dma = getattr(nc, w_engine)
                    for i in range(w_split):
                        w_dma.dma_start(
                            whs[i][:], wt[:, ns, i * KHALF : (i + 1) * KHALF]
                        )
                    pts = [
                        psum_pool.tile([P, MT], mybir.dt.float32, name=f"pt{i}", tag="pt")
                        for i in range(MTILES)
                    ]
                    # ko outer / mt inner: consecutive matmuls share the
                    # stationary W block, halving LDWEIGHTS traffic.
                    for ko in range(KO):
                        for mt in range(MTILES):
                            nc.tensor.matmul(
                                pts[mt][:],
                                whs[ko // KHALF][:, ko % KHALF],
                                xres[:, ko, mt * MT : (mt + 1) * MT],
                                start=(ko == 0),
                                stop=(ko == KO - 1),
                            )
                    for mt in range(MTILES):
                        ot_sb = out_pool.tile([P, MT], mybir.dt.float32)
                        nc.scalar.add(ot_sb[:], pts[mt][:], bias_sb[:, ns : ns + 1])
                        out_dma = nc.gpsimd if out_engine == "gpsimd" else nc.sync
                        out_dma.dma_start(
                            ot[:, ns, mt * MT : (mt + 1) * MT], ot_sb[:]
                        )

            if repeats == 1:
                body()
            else:
                # Unroll several complete forward passes per For_i trip:
                # For_i inserts an all-engine barrier at each back edge, so
                # unrolling amortizes the barrier + pipeline refill, and the
                # x_bufs=2 ping-pong lets body k+1's x DMA stream overlap
                # body k's tail matmuls.
                u = math.gcd(unroll, repeats)
                with tc.For_i(0, repeats // u, 1) as iv:
                    for _ in range(u):
                        body(iv)

    nc.compile()
    if dedupe_ldw:
        # Halves the LDWEIGHTS count (1024 -> ~518).  Measured on HW:
        # no speedup (277.7us vs 272.6us best without) -- the weight-switch
        # cost is not the LDWEIGHTS instruction stream; left off.
        _dedupe_ldweights(nc)
    return nc


def _dedupe_ldweights(nc) -> int:
    """Drop redundant back-to-back InstLdweights from the compiled PE streams.

    The lowering splits every matmul into its own LDWEIGHTS + MATMUL
    (ldweights=False) pair, even when consecutive matmuls share the same
    stationary block, and the PE executes each LDWEIGHTS serially
    (~50 ns per 128-col load, measured).  The PE array keeps its weights
    across matmuls that don't self-load, so an LDWEIGHTS whose operand is
    identical to the previous one (and that carries no semaphore wait or
    update) is pure overhead.  Removing them saves ~matmul_count/2 loads.
    """
    removed = 0
    for fn in nc.m.functions:
        for blk in fn.blocks:
            prev_key = None
            keep = []
            for inst in blk.instructions:
                if getattr(inst, "engine", None) != mybir.EngineType.PE:
                    keep.append(inst)
                    continue
                nm = inst.__class__.__name__
                if nm == "InstLdweights":
                    key = (
                        str(inst.ins[0]),
                        inst.is_transpose,
                        inst.tile_position,
                        inst.perf_mode,
                        str(inst.tile_size),
                    )
                    if (
                        key == prev_key
                        and not inst.has_wait()
                        and not inst.has_update()
                    ):
                        removed += 1
                        continue
                    prev_key = key
                    keep.append(inst)
                elif nm == "InstMatmult" and inst.ldweights is False:
                    # does not disturb the loaded weights
                    keep.append(inst)
                else:
                    # anything else on PE: conservatively invalidate
                    prev_key = None
                    keep.append(inst)
            if removed:
                blk.instructions[:] = keep
    return removed


def _shard_inputs(x: np.ndarray, W: np.ndarray, b: np.ndarray, mode: str = "bf16"):
    """Host-side shard + retile into the DMA-friendly layouts."""
    import ml_dtypes

    np_dt = ml_dtypes.bfloat16 if mode == "bf16" else np.float32
    in_maps = []
    wt_cache = {}
    bs_cache = {}
    for c in range(N_CORES):
        q, h = divmod(c, N_SHARD)
        xs = x[q * M : (q + 1) * M]  # [M, IN]
        xt = xs.reshape(M, KO, P).transpose(2, 1, 0).astype(np_dt, order="C")
        if (h,) not in wt_cache:
            Ws = W[h * N : (h + 1) * N]  # [N, IN]
            wt_cache[(h,)] = (
                Ws.reshape(NSUB, P, KO, P)
                .transpose(3, 0, 2, 1)
                .astype(np_dt, order="C")
            )
            bs_cache[(h,)] = np.ascontiguousarray(
                b[h * N : (h + 1) * N].reshape(NSUB, P).T
            )
        in_maps.append({"xt": xt, "wt": wt_cache[(h,)], "bs": bs_cache[(h,)]})
    return in_maps


def _assemble(results) -> np.ndarray:
    out = np.empty((BATCH, OUT_DIM), dtype=np.float32)
    for c in range(N_CORES):
        q, h = divmod(c, N_SHARD)
        ot = results[c]["ot"]  # [P, NSUB, M]
        block = ot.transpose(2, 1, 0).reshape(M, N)
        out[q * M : (q + 1) * M, h * N : (h + 1) * N] = block
    return out


def kernel(x: np.ndarray, W: np.ndarray, b: np.ndarray) -> np.ndarray:
    x = np.asarray(x, dtype=np.float32)
    W = np.asarray(W, dtype=np.float32)
    b = np.asarray(b, dtype=np.float32)
    assert x.shape == (BATCH, IN_DIM) and W.shape == (OUT_DIM, IN_DIM)

    if "nc" not in _CACHE:
        _CACHE["nc"] = _build_program()
    nc = _CACHE["nc"]

    in_maps = _shard_inputs(x, W, b)
    res = run_bass_kernel_spmd(nc, in_maps, list(range(N_CORES)))
    return _assemble(res.results)


if __name__ == "__main__":
    rng = np.random.default_rng(0)
    x = rng.standard_normal((BATCH, IN_DIM), dtype=np.float32)
    W = rng.uniform(-1 / 64, 1 / 64, size=(OUT_DIM, IN_DIM)).astype(np.float32)
    b = rng.uniform(-1 / 64, 1 / 64, size=(OUT_DIM,)).astype(np.float32)
    got = kernel(x, W, b)
    exp = x @ W.T + b
    scale = np.abs(exp).max()
    print("absmax err:", np.abs(got - exp).max(), "scale:", scale)

